# revision 29
# baseline (speedup 1.0000x reference)
"""Trainium2 Bass kernel for nn_ExponentialSmoothingAttention.

Reference computes, per head h with a_h = sigmoid(alpha_h):
    out[b, t, (h,d)] = sum_{k>=0} a_h * (1-a_h)^k * Vext[b, t+k, (h,d)]
where Vext = concat([v0 broadcast, V], time) (reversed-time EMA via FFT conv).

The geometric weights decay fast, so this is a 6-tap FIR along time,
computed as a banded-Toeplitz matmul on the PE array: blocks of 123 output
rows from 128 input rows (123 + 5 halo), stationary weight W[j, i] = c_{j-i}.

The problem is HBM-bound, so both directions stream int8 (4x fewer bytes
than fp32):
- input: host quantizes V per-super (shared scale s_s = max|X_super|/127
  across cores) into the halo-blocked int8 layout; the device upcasts
  int8 -> bf16 (exact small integers) with one full-tile Vector tensor_copy
  per super (237 G elem/s in the DVE fast mode; a partial-width dst AP
  halves that), then runs bf16 matmuls with fp32 PSUM.
- output: the PSUM->SBUF evacuation copy applies scale s_s/s_y as a baked
  immediate and saturate-casts to int8 (Vector tensor_scalar_mul / Scalar
  activation-Copy, 1:3 by measured rates); host dequantizes by the global
  s_y = 4.3*sigma_y/127 (the ~1e-5 clip tail saturates harmlessly).
Measured end-to-end rel err 1.6e-2 vs the 2e-2 gate (deterministic seed).

DMA structure (measured on trn2):
- reads: sync HWDGE ring, one DMA per super, 128 descriptors of G*CPC
  bytes; saturates all 16 SDMA engines (~390 GB/s).
- writes: SWDGE (gpsimd) in 6 contiguous-partition chunks per super
  (HWDGE writes only engage 3-6 engines; SWDGE chunks cover all 16).
  DRAM rows are interleaved (chunk c's i-th partition -> row c + 6i) so
  consecutive 8 KB descriptors are never DRAM-adjacent and can't
  re-aggregate into slow >8 KB bursts; host inverts the permutation.
  Q7 descriptor-gen is ~1.2us per dma_start and paces the drain tail.
Remaining fixed costs: ~2.5us dispatch-to-first-byte and a ~10us
framework exit sweep (each sequencer serially rechecks its share of all
256 kernel semaphores; program-independent).

Sharding: 8 cores = (batch b in 0..3) x (channel half in 0..1); each core
processes [8192 time, 512 channels].  No cross-core communication.
"""

import numpy as np
import ml_dtypes

import concourse.bacc as bacc
import concourse.mybir as mybir
import concourse.tile as tile
from concourse.ap import AP
from concourse.bass_utils import run_bass_kernel_spmd

B, L, DM, NH, DH = 4, 8192, 1024, 16, 64
CPC = 512                      # channels per core (DM / 2)
W_TAPS = 6                     # FIR window; (1-a)^6 ~ 2.9e-3 rel truncation
M_BLK = 128 - (W_TAPS - 1)     # 123 output rows per matmul block
K_BLK = 128                    # input rows per block (123 + 5 halo)
N_BLOCKS = -(-L // M_BLK)      # 67
X_ROWS = M_BLK * (N_BLOCKS - 1) + K_BLK   # v0 + 8192 V rows + zero pad

BF16 = mybir.dt.bfloat16
I8 = mybir.dt.int8
NP_BF16 = ml_dtypes.bfloat16

# first super small so the first compute starts one short DMA after start
SUPERS = [(0, 3), (3, 16), (19, 16), (35, 16), (51, 16)]

# supers are processed in half-tiles: reads/upcasts/copies/writes all run
# at half-super granularity so matmuls start after a ~2us upcast and the
# final write unit (desc-gen + drain) is half-sized


def _halves(G):
    return [(0, G)] if G <= 4 else [(0, G // 2), (G // 2, G - G // 2)]


# output DRAM slabs per (super, half), 8 KB aligned.  A slab holds Y_ROWS
# rows of Gh*CPC int8; the write interleave maps chunk c's i-th partition
# to DRAM row c + Y_CHUNKS*i (injective, max row Y_ROWS-1); the host
# applies the inverse permutation.
Y_CHUNKS = 3
Y_STEP = -(-M_BLK // Y_CHUNKS)     # 41
Y_ROWS = (Y_CHUNKS - 1) + Y_CHUNKS * (Y_STEP - 1) + 1   # 123
_Y_BASE = {}                       # (si, h0) -> element offset
_off = 0
for _si, (_g0, _G) in enumerate(SUPERS):
    for _h0, _Gh in _halves(_G):
        _Y_BASE[(_si, _h0)] = _off
        _off += -(-(_Gh * CPC * Y_ROWS) // 8192) * 8192
Y_TOTAL = _off

# PSUM-tile evacuation engine cycle: Vector takes 1 in 4 (it also runs
# the upcasts; Scalar's ACT copies are the cheaper per-element path)
COPY_CYCLE = ("v", "s", "s", "s")

S_Y_SIGMA = 4.3                # output scale clip point (in sigma_y)

TRACE = False                  # test harness flips this for profiling
LAST_RESULT = None             # BassKernelResults of the most recent run

_PROGRAM_CACHE = None          # (lams_key, nc)


def _f32(x):
    return np.ascontiguousarray(x, dtype=np.float32)


def _build_program(lams):
    """lams[s] = per-super immediate scale for the PSUM->int8 copies."""
    nc = bacc.Bacc("TRN2")
    x = nc.dram_tensor("x", [K_BLK * N_BLOCKS * CPC], I8, kind="ExternalInput")
    w = nc.dram_tensor("w", [K_BLK, M_BLK], BF16, kind="ExternalInput")
    y = nc.dram_tensor("y", [Y_TOTAL], I8, kind="ExternalOutput")

    with tile.TileContext(nc) as tc:
        with (
            tc.tile_pool(name="wp", bufs=1) as wp,
            tc.tile_pool(name="xin", bufs=5) as xin,
            tc.tile_pool(name="xbf", bufs=4) as xbf,
            tc.tile_pool(name="yout", bufs=4) as yout,
            tc.tile_pool(name="ps", bufs=4, space=bacc.bass.MemorySpace.PSUM) as ps,
        ):
            wt = wp.tile([K_BLK, M_BLK], BF16)
            nc.scalar.dma_start(wt[:], w[:])

            cyc = 0
            for s, (g0, G) in enumerate(SUPERS):
                Wc = G * CPC
                lam = float(lams[s])
                # read + upcast the super in TWO half-tiles: matmuls of the
                # first half start after a ~2us upcast instead of waiting
                # for the whole super's ~4us upcast (PE boundary bubble).
                # Separate full tiles keep the DVE upcast in its fast mode
                # (a partial-width AP was measured to halve the rate).
                xbs = []
                for h0, Gh in _halves(G):
                    Wh = Gh * CPC
                    xt = xin.tile([K_BLK, Wh], I8, tag="xt")
                    src = AP(x, (g0 + h0) * CPC * K_BLK,
                             [[Wh, K_BLK], [1, Wh]])
                    nc.sync.dma_start(xt[:], src)
                    xb = xbf.tile([K_BLK, Wh], BF16, tag="xb")
                    nc.vector.tensor_copy(xb[:], xt[:])
                    xbs.append((h0, Gh, xb))

                for h0, Gh, xb in xbs:
                    Wh = Gh * CPC
                    yt = yout.tile([M_BLK, Wh], I8, tag="yt")
                    g = 0
                    while g < Gh:
                        gp = min(2, Gh - g)   # 2 blocks = one 2-bank PSUM tile
                        pt = ps.tile([M_BLK, gp * CPC], mybir.dt.float32,
                                     tag="pt")
                        for j in range(gp):
                            nc.tensor.matmul(
                                pt[:, j * CPC:(j + 1) * CPC], wt[:],
                                xb[:, (g + j) * CPC:(g + j + 1) * CPC],
                                start=True, stop=True)
                        dst = yt[:, g * CPC:(g + gp) * CPC]
                        if COPY_CYCLE[cyc % len(COPY_CYCLE)] == "v":
                            nc.vector.tensor_scalar_mul(dst, pt[:], lam)
                        else:
                            nc.scalar.activation(
                                dst, pt[:],
                                mybir.ActivationFunctionType.Copy,
                                bias=0.0, scale=lam)
                        cyc += 1
                        g += gp

                    # store the half via SWDGE in 3 chunks of contiguous
                    # SBUF partitions (SWDGE chunks cover all 16 SDMA
                    # engines; HWDGE writes engage only 3-6).  DRAM row for
                    # chunk c's i-th partition is c + 3*i, so consecutive
                    # burst descriptors are never DRAM-adjacent and can't
                    # re-aggregate into slower bigger bursts.
                    base = _Y_BASE[(s, h0)]
                    for c, p0 in enumerate(range(0, M_BLK, Y_STEP)):
                        pn = min(Y_STEP, M_BLK - p0)
                        dst = AP(y, base + c * Wh,
                                 [[Y_CHUNKS * Wh, pn], [1, Wh]])
                        nc.gpsimd.dma_start(dst, yt[p0:p0 + pn, :])

    nc.compile()
    return nc


def _fir_coeffs(a64):
    k = np.arange(W_TAPS, dtype=np.float64)
    return a64 * (1.0 - a64) ** k


def _weight_matrix(a64):
    c = _fir_coeffs(a64).astype(np.float32)
    wmat = np.zeros((K_BLK, M_BLK), dtype=np.float32)
    i = np.arange(M_BLK)
    for k in range(W_TAPS):
        wmat[i + k, i] = c[k]
    return wmat


def _numpy_fallback(V, alpha, v0):
    # General per-head path (never hit for the oracle's uniform alpha).
    a = 1.0 / (1.0 + np.exp(-alpha.astype(np.float64)))       # [NH]
    taps = 48
    k = np.arange(taps, dtype=np.float64)
    c = a[:, None] * (1.0 - a[:, None]) ** k[None, :]         # [NH, taps]
    c_ch = np.repeat(c, DH, axis=0)                           # [DM, taps]
    v0row = v0.reshape(1, DM).astype(np.float64)
    out = np.zeros((B, L, DM), dtype=np.float64)
    for b in range(B):
        vext = np.concatenate(
            [v0row, V[b].astype(np.float64), np.zeros((taps, DM))], axis=0)
        for kk in range(taps):
            out[b] += c_ch[:, kk][None, :] * vext[kk:kk + L]
    return out.astype(np.float32)


def kernel(V, alpha, v0):
    global _PROGRAM_CACHE, LAST_RESULT
    V = _f32(V)
    alpha = _f32(alpha).reshape(-1)
    v0 = _f32(v0)

    a64 = 1.0 / (1.0 + np.exp(-alpha.astype(np.float64)))
    if not np.allclose(a64, a64[0], rtol=0, atol=1e-12):
        return _numpy_fallback(V, alpha, v0)

    c = _fir_coeffs(a64[0])
    sigma = float(V.std(dtype=np.float64))
    sigma_y = float(np.sqrt((c ** 2).sum())) * sigma
    s_y = S_Y_SIGMA * sigma_y / 127.0

    wmat = _weight_matrix(a64[0]).astype(NP_BF16)
    v0_flat = v0.reshape(DM)

    # pass 1: halo-block each core's input; per-super scale shared across
    # cores (the SPMD program bakes one immediate per super)
    x_blks = []
    smax = [0.0] * len(SUPERS)
    for core in range(8):
        b, half = divmod(core, 2)
        ch = slice(half * CPC, (half + 1) * CPC)
        X = np.zeros((X_ROWS, CPC), dtype=np.float32)
        X[0] = v0_flat[ch]
        X[1:L + 1] = V[b, :, ch]
        # halo-block: X_blk[i, g*CPC + c] = X[M_BLK*g + i, c]
        sv = np.lib.stride_tricks.as_strided(
            X, shape=(N_BLOCKS, K_BLK, CPC),
            strides=(M_BLK * X.strides[0], X.strides[0], X.strides[1]))
        X_blk = np.ascontiguousarray(sv.transpose(1, 0, 2)).reshape(
            K_BLK, N_BLOCKS * CPC)
        x_blks.append(X_blk)
        for si, (g0, G) in enumerate(SUPERS):
            m = float(np.abs(X_blk[:, g0 * CPC:(g0 + G) * CPC]).max())
            smax[si] = max(smax[si], m)

    s_sup = [max(m, 1e-30) / 127.0 for m in smax]
    all_lams = [s / s_y for s in s_sup]

    in_maps = []
    for core in range(8):
        X_blk = x_blks[core]
        x_flat = np.empty(K_BLK * N_BLOCKS * CPC, dtype=np.int8)
        for si, (g0, G) in enumerate(SUPERS):
            # pack each read half-tile contiguously ([K_BLK, Gh*CPC]
            # row-major at (g0+h0)*CPC*K_BLK), matching the device APs
            for h0, Gh in _halves(G):
                b0 = g0 + h0
                slab = X_blk[:, b0 * CPC:(b0 + Gh) * CPC]
                q = np.rint(slab * (1.0 / s_sup[si])).astype(np.int8)
                base = b0 * CPC * K_BLK
                x_flat[base:base + Gh * CPC * K_BLK] = q.reshape(-1)
        in_maps.append({"x": x_flat, "w": wmat})
    del x_blks

    key = tuple(round(l, 12) for l in all_lams)
    if _PROGRAM_CACHE is None or _PROGRAM_CACHE[0] != key:
        _PROGRAM_CACHE = (key, _build_program(all_lams))
    nc = _PROGRAM_CACHE[1]

    kwargs = {}
    if TRACE:
        kwargs = {"trace": True, "trace_cores": list(range(8))}
    LAST_RESULT = run_bass_kernel_spmd(
        nc, in_maps, core_ids=list(range(8)), **kwargs)

    # partition p of a half's yt lives at DRAM row (p//step)+chunks*(p%step)
    p = np.arange(M_BLK)
    rowperm = (p // Y_STEP) + Y_CHUNKS * (p % Y_STEP)

    out = np.empty((B, L, DM), dtype=np.float32)
    for core in range(8):
        b, half = divmod(core, 2)
        y_raw = np.asarray(LAST_RESULT.results[core]["y"])  # flat int8
        y_blk = np.empty((M_BLK, N_BLOCKS * CPC), dtype=np.float32)
        for si, (g0, G) in enumerate(SUPERS):
            for h0, Gh in _halves(G):
                base = _Y_BASE[(si, h0)]
                rows = y_raw[base:base + Gh * CPC * Y_ROWS].reshape(
                    Y_ROWS, Gh * CPC)
                b0 = g0 + h0
                y_blk[:, b0 * CPC:(b0 + Gh) * CPC] = rows[rowperm].astype(
                    np.float32)
        y_flat = y_blk.reshape(M_BLK, N_BLOCKS, CPC).transpose(1, 0, 2).reshape(
            M_BLK * N_BLOCKS, CPC)
        out[b, :, half * CPC:(half + 1) * CPC] = y_flat[:L] * np.float32(s_y)
    return out


# revision 31
# speedup vs baseline: 1.3643x; 1.3643x over previous
"""Trainium2 Bass kernel for nn_ExponentialSmoothingAttention.

Reference computes, per head h with a_h = sigmoid(alpha_h):
    out[b, t, (h,d)] = sum_{k>=0} a_h * (1-a_h)^k * Vext[b, t+k, (h,d)]
where Vext = concat([v0 broadcast, V], time) (reversed-time EMA via FFT conv).

The geometric weights decay fast, so this is a 6-tap FIR along time,
computed as a banded-Toeplitz matmul on the PE array: blocks of 123 output
rows from 128 input rows (123 + 5 halo), stationary weight W[j, i] = c_{j-i}.

The problem is HBM-bound, so both directions stream int8 (4x fewer bytes
than fp32):
- input: host quantizes V per-super (shared scale s_s = max|X_super|/127
  across cores) into the halo-blocked int8 layout; the device upcasts
  int8 -> bf16 (exact small integers) with one full-tile Vector tensor_copy
  per super (237 G elem/s in the DVE fast mode; a partial-width dst AP
  halves that), then runs bf16 matmuls with fp32 PSUM.
- output: the PSUM->SBUF evacuation copy applies scale s_s/s_y as a baked
  immediate and saturate-casts to int8 (Vector tensor_scalar_mul / Scalar
  activation-Copy, 1:3 by measured rates); host dequantizes by the global
  s_y = 4.3*sigma_y/127 (the ~1e-5 clip tail saturates harmlessly).
Measured end-to-end rel err 1.6e-2 vs the 2e-2 gate (deterministic seed).

DMA structure (measured on trn2):
- reads: sync HWDGE ring, one DMA per super, 128 descriptors of G*CPC
  bytes; saturates all 16 SDMA engines (~390 GB/s).
- writes: SWDGE (gpsimd) in 6 contiguous-partition chunks per super
  (HWDGE writes only engage 3-6 engines; SWDGE chunks cover all 16).
  DRAM rows are interleaved (chunk c's i-th partition -> row c + 6i) so
  consecutive 8 KB descriptors are never DRAM-adjacent and can't
  re-aggregate into slow >8 KB bursts; host inverts the permutation.
  Q7 descriptor-gen is ~1.2us per dma_start and paces the drain tail.
Remaining fixed costs: ~2.5us dispatch-to-first-byte and a ~10us
framework exit sweep (each sequencer serially rechecks its share of all
256 kernel semaphores; program-independent).

Sharding: 8 cores = (batch b in 0..3) x (channel half in 0..1); each core
processes [8192 time, 512 channels].  No cross-core communication.
"""

import numpy as np
import ml_dtypes

import concourse.bacc as bacc
import concourse.mybir as mybir
import concourse.tile as tile
from concourse.ap import AP
from concourse.bass_utils import run_bass_kernel_spmd

B, L, DM, NH, DH = 4, 8192, 1024, 16, 64
CPC = 512                      # channels per core (DM / 2)
W_TAPS = 6                     # FIR window; (1-a)^6 ~ 2.9e-3 rel truncation
M_BLK = 128 - (W_TAPS - 1)     # 123 output rows per matmul block
K_BLK = 128                    # input rows per block (123 + 5 halo)
N_BLOCKS = -(-L // M_BLK)      # 67
X_ROWS = M_BLK * (N_BLOCKS - 1) + K_BLK   # v0 + 8192 V rows + zero pad

BF16 = mybir.dt.bfloat16
I8 = mybir.dt.int8
NP_BF16 = ml_dtypes.bfloat16

# first super small so the first compute starts one short DMA after start
SUPERS = [(0, 3), (3, 16), (19, 16), (35, 16), (51, 16)]

# supers are processed in half-tiles: reads/upcasts/copies/writes all run
# at half-super granularity so matmuls start after a ~2us upcast and the
# final write unit (desc-gen + drain) is half-sized


def _halves(G):
    return [(0, G)] if G <= 4 else [(0, G // 2), (G // 2, G - G // 2)]


# output DRAM slabs per super, 8 KB aligned.  A slab holds Y_ROWS rows of
# G*CPC int8; the write interleave maps chunk c's i-th partition to DRAM
# row c + Y_CHUNKS*i (injective, max row Y_ROWS-1); the host applies the
# inverse permutation.  6 chunks x 21 partitions x 8 KB rows is a measured
# sweet spot: 3- and 4-chunk variants and per-half (4 KB-row) slabs all
# regressed 20-30%.
Y_CHUNKS = 6
Y_STEP = -(-M_BLK // Y_CHUNKS)     # 21
Y_ROWS = (Y_CHUNKS - 1) + Y_CHUNKS * (Y_STEP - 1) + 1   # 126
_Y_BASES = []
_off = 0
for _g0, _G in SUPERS:
    _Y_BASES.append(_off)
    _off += -(-(_G * CPC * Y_ROWS) // 8192) * 8192
Y_TOTAL = _off

# PSUM-tile evacuation engine cycle: Vector takes 1 in 4 (it also runs
# the upcasts; Scalar's ACT copies are the cheaper per-element path)
COPY_CYCLE = ("v", "s", "s", "s")

S_Y_SIGMA = 4.3                # output scale clip point (in sigma_y)

TRACE = False                  # test harness flips this for profiling
LAST_RESULT = None             # BassKernelResults of the most recent run

_PROGRAM_CACHE = None          # (lams_key, nc)


def _f32(x):
    return np.ascontiguousarray(x, dtype=np.float32)


def _build_program(lams):
    """lams[s] = per-super immediate scale for the PSUM->int8 copies."""
    nc = bacc.Bacc("TRN2")
    x = nc.dram_tensor("x", [K_BLK * N_BLOCKS * CPC], I8, kind="ExternalInput")
    w = nc.dram_tensor("w", [K_BLK, M_BLK], BF16, kind="ExternalInput")
    y = nc.dram_tensor("y", [Y_TOTAL], I8, kind="ExternalOutput")

    with tile.TileContext(nc) as tc:
        with (
            tc.tile_pool(name="wp", bufs=1) as wp,
            tc.tile_pool(name="xin", bufs=8) as xin,
            tc.tile_pool(name="xbf", bufs=6) as xbf,
            tc.tile_pool(name="yout", bufs=5) as yout,
            tc.tile_pool(name="ps", bufs=4, space=bacc.bass.MemorySpace.PSUM) as ps,
        ):
            wt = wp.tile([K_BLK, M_BLK], BF16)
            nc.scalar.dma_start(wt[:], w[:])

            cyc = 0
            for s, (g0, G) in enumerate(SUPERS):
                Wc = G * CPC
                lam = float(lams[s])
                # read + upcast the super in TWO half-tiles: matmuls of the
                # first half start after a ~2us upcast instead of waiting
                # for the whole super's ~4us upcast (PE boundary bubble).
                # Separate full tiles keep the DVE upcast in its fast mode
                # (a partial-width AP was measured to halve the rate).
                xbs = []
                for h0, Gh in _halves(G):
                    Wh = Gh * CPC
                    xt = xin.tile([K_BLK, Wh], I8, tag="xt")
                    src = AP(x, (g0 + h0) * CPC * K_BLK,
                             [[Wh, K_BLK], [1, Wh]])
                    nc.sync.dma_start(xt[:], src)
                    xb = xbf.tile([K_BLK, Wh], BF16, tag="xb")
                    nc.vector.tensor_copy(xb[:], xt[:])
                    xbs.append((h0, Gh, xb))

                yt = yout.tile([M_BLK, Wc], I8, tag="yt")
                for h0, Gh, xb in xbs:
                    g = 0
                    while g < Gh:
                        gp = min(2, Gh - g)   # 2 blocks = one 2-bank PSUM tile
                        pt = ps.tile([M_BLK, gp * CPC], mybir.dt.float32,
                                     tag="pt")
                        for j in range(gp):
                            nc.tensor.matmul(
                                pt[:, j * CPC:(j + 1) * CPC], wt[:],
                                xb[:, (g + j) * CPC:(g + j + 1) * CPC],
                                start=True, stop=True)
                        dst = yt[:, (h0 + g) * CPC:(h0 + g + gp) * CPC]
                        if COPY_CYCLE[cyc % len(COPY_CYCLE)] == "v":
                            nc.vector.tensor_scalar_mul(dst, pt[:], lam)
                        else:
                            nc.scalar.activation(
                                dst, pt[:],
                                mybir.ActivationFunctionType.Copy,
                                bias=0.0, scale=lam)
                        cyc += 1
                        g += gp

                # store per super via SWDGE in 6 chunks of contiguous SBUF
                # partitions (SWDGE chunks cover all 16 SDMA engines; HWDGE
                # writes engage only 3-6).  DRAM row for chunk c's i-th
                # partition is c + 6*i, so consecutive 8 KB descriptors are
                # never DRAM-adjacent and can't re-aggregate into slow
                # >8 KB bursts.
                base = _Y_BASES[s]
                for c, p0 in enumerate(range(0, M_BLK, Y_STEP)):
                    pn = min(Y_STEP, M_BLK - p0)
                    dst = AP(y, base + c * Wc,
                             [[Y_CHUNKS * Wc, pn], [1, Wc]])
                    nc.gpsimd.dma_start(dst, yt[p0:p0 + pn, :])

    nc.compile()
    return nc


def _fir_coeffs(a64):
    k = np.arange(W_TAPS, dtype=np.float64)
    return a64 * (1.0 - a64) ** k


def _weight_matrix(a64):
    c = _fir_coeffs(a64).astype(np.float32)
    wmat = np.zeros((K_BLK, M_BLK), dtype=np.float32)
    i = np.arange(M_BLK)
    for k in range(W_TAPS):
        wmat[i + k, i] = c[k]
    return wmat


def _numpy_fallback(V, alpha, v0):
    # General per-head path (never hit for the oracle's uniform alpha).
    a = 1.0 / (1.0 + np.exp(-alpha.astype(np.float64)))       # [NH]
    taps = 48
    k = np.arange(taps, dtype=np.float64)
    c = a[:, None] * (1.0 - a[:, None]) ** k[None, :]         # [NH, taps]
    c_ch = np.repeat(c, DH, axis=0)                           # [DM, taps]
    v0row = v0.reshape(1, DM).astype(np.float64)
    out = np.zeros((B, L, DM), dtype=np.float64)
    for b in range(B):
        vext = np.concatenate(
            [v0row, V[b].astype(np.float64), np.zeros((taps, DM))], axis=0)
        for kk in range(taps):
            out[b] += c_ch[:, kk][None, :] * vext[kk:kk + L]
    return out.astype(np.float32)


def kernel(V, alpha, v0):
    global _PROGRAM_CACHE, LAST_RESULT
    V = _f32(V)
    alpha = _f32(alpha).reshape(-1)
    v0 = _f32(v0)

    a64 = 1.0 / (1.0 + np.exp(-alpha.astype(np.float64)))
    if not np.allclose(a64, a64[0], rtol=0, atol=1e-12):
        return _numpy_fallback(V, alpha, v0)

    c = _fir_coeffs(a64[0])
    sigma = float(V.std(dtype=np.float64))
    sigma_y = float(np.sqrt((c ** 2).sum())) * sigma
    s_y = S_Y_SIGMA * sigma_y / 127.0

    wmat = _weight_matrix(a64[0]).astype(NP_BF16)
    v0_flat = v0.reshape(DM)

    # pass 1: halo-block each core's input; per-super scale shared across
    # cores (the SPMD program bakes one immediate per super)
    x_blks = []
    smax = [0.0] * len(SUPERS)
    for core in range(8):
        b, half = divmod(core, 2)
        ch = slice(half * CPC, (half + 1) * CPC)
        X = np.zeros((X_ROWS, CPC), dtype=np.float32)
        X[0] = v0_flat[ch]
        X[1:L + 1] = V[b, :, ch]
        # halo-block: X_blk[i, g*CPC + c] = X[M_BLK*g + i, c]
        sv = np.lib.stride_tricks.as_strided(
            X, shape=(N_BLOCKS, K_BLK, CPC),
            strides=(M_BLK * X.strides[0], X.strides[0], X.strides[1]))
        X_blk = np.ascontiguousarray(sv.transpose(1, 0, 2)).reshape(
            K_BLK, N_BLOCKS * CPC)
        x_blks.append(X_blk)
        for si, (g0, G) in enumerate(SUPERS):
            m = float(np.abs(X_blk[:, g0 * CPC:(g0 + G) * CPC]).max())
            smax[si] = max(smax[si], m)

    s_sup = [max(m, 1e-30) / 127.0 for m in smax]
    all_lams = [s / s_y for s in s_sup]

    in_maps = []
    for core in range(8):
        X_blk = x_blks[core]
        x_flat = np.empty(K_BLK * N_BLOCKS * CPC, dtype=np.int8)
        for si, (g0, G) in enumerate(SUPERS):
            # pack each read half-tile contiguously ([K_BLK, Gh*CPC]
            # row-major at (g0+h0)*CPC*K_BLK), matching the device APs
            for h0, Gh in _halves(G):
                b0 = g0 + h0
                slab = X_blk[:, b0 * CPC:(b0 + Gh) * CPC]
                q = np.rint(slab * (1.0 / s_sup[si])).astype(np.int8)
                base = b0 * CPC * K_BLK
                x_flat[base:base + Gh * CPC * K_BLK] = q.reshape(-1)
        in_maps.append({"x": x_flat, "w": wmat})
    del x_blks

    key = tuple(round(l, 12) for l in all_lams)
    if _PROGRAM_CACHE is None or _PROGRAM_CACHE[0] != key:
        _PROGRAM_CACHE = (key, _build_program(all_lams))
    nc = _PROGRAM_CACHE[1]

    kwargs = {}
    if TRACE:
        kwargs = {"trace": True, "trace_cores": list(range(8))}
    LAST_RESULT = run_bass_kernel_spmd(
        nc, in_maps, core_ids=list(range(8)), **kwargs)

    # partition p of a half's yt lives at DRAM row (p//step)+chunks*(p%step)
    p = np.arange(M_BLK)
    rowperm = (p // Y_STEP) + Y_CHUNKS * (p % Y_STEP)

    out = np.empty((B, L, DM), dtype=np.float32)
    for core in range(8):
        b, half = divmod(core, 2)
        y_raw = np.asarray(LAST_RESULT.results[core]["y"])  # flat int8
        y_blk = np.empty((M_BLK, N_BLOCKS * CPC), dtype=np.float32)
        for si, (g0, G) in enumerate(SUPERS):
            base = _Y_BASES[si]
            rows = y_raw[base:base + G * CPC * Y_ROWS].reshape(
                Y_ROWS, G * CPC)
            y_blk[:, g0 * CPC:(g0 + G) * CPC] = rows[rowperm].astype(
                np.float32)
        y_flat = y_blk.reshape(M_BLK, N_BLOCKS, CPC).transpose(1, 0, 2).reshape(
            M_BLK * N_BLOCKS, CPC)
        out[b, :, half * CPC:(half + 1) * CPC] = y_flat[:L] * np.float32(s_y)
    return out
